# revision 31
# baseline (speedup 1.0000x reference)
"""MoE top-2 routing kernel for 8 Trainium2 NeuronCores.

Problem (hardcoded shapes): x [64,8,2048] f32, gate_w [2048,8] f32,
w1/w3 [8,2048,4096] f32, w2 [8,4096,2048] f32, top_k=2.

Strategy (expert parallelism):
  - Host computes the gate (512x8 logits, top-2, softmax) exactly as the
    reference does -- ~17 MFLOP, negligible.
  - Tokens are dispatched per expert (gathered + padded to capacity C),
    one expert per NeuronCore.  Each core runs the SwiGLU FFN for its
    expert over its C token slots:
        outT = w2^T @ (silu(w1^T @ xT) * (w3^T @ xT))
    with all matmuls laid out [K, M]/[K, N] so no on-device transposes
    are needed (tokens are the moving free dim).
  - Combine weights are folded into the host-side scatter-add.

Precision plan (rel-err budget 2e-2; predictions from a bit-exact numpy
model of the device pipeline, which has matched HW to 4 digits on every
run so far):
  - activations bf16
  - w3 all fp8 e3m4, per-column scales folded into w2's rows (exact)
  - w2: 2 of 4 output-column groups fp8 e3m4, per-column scales folded
    into the host combine (exact)
  - w1: 8 of 16 column groups fp8 e3m4 with a constant x128 pre-scale
    (clipped at +-15.4); the 1/128 un-scale folds into the silu's
    `scale` operand (exact, free)
  Bytes are balanced so stage 1 (w1+w3, ~21MB) matches its ~64us of PE
  work and stage 2 (w2, ~12.6MB) only slightly exceeds its ~34us.
  Predicted rel err 1.908e-2.

Perf notes (measured):
  - one HWDGE ring (sync) sustains ~360 GB/s streaming; ALL DMAs go on
    it.  The scalar engine must issue NO DMAs: its ACTIVATEs wait on PE
    progress and, via strict FIFO + shared semaphore lanes, would pace
    the whole DMA subsystem at PE speed.
  - deep weight pools so DMA issue is never gated by PE progress.
  - C = tokens padded to a multiple of 16 (>=128): C must stay 16-
    aligned or half the xt k-slices are 8B-misaligned in SBUF and every
    matmul pays ~+15ns streaming (measured at C=140).
  - stage-1 groups of G1=2 m-tiles (4 PSUM banks) and stage-2 groups of
    G2=4 (4 banks): consecutive groups ping-pong PSUM banks.
  - 25 warmup matmuls on zeros keep the PE busy while the first weight
    blocks land, so HAM reaches K=8/8 before real work and never
    re-throttles.
"""

import numpy as np

B, S, D, F, E = 64, 8, 2048, 4096, 8
T = B * S  # 512 tokens
P = 128
KD = D // P   # 16 k-tiles, D contraction
KF = F // P   # 32 k-tiles, F contraction
MF = F // P   # 32 m-tiles, stage 1
MD = D // P   # 16 m-tiles, stage 2
G1 = 2        # stage-1 m-tiles per group (2 gate + 2 up = 4 PSUM banks)
G2 = 4        # stage-2 m-tiles per group (4 PSUM banks)
NG1 = MF // G1   # 16 stage-1 groups
NG2 = MD // G2   # 4 stage-2 groups

W1_NFP8 = 8     # stage-1 groups (of NG1) whose w1 is fp8 (the last ones)
W3_NFP8 = 16    # stage-1 groups (of NG1) whose w3 is fp8
W2_NFP8G = 2    # stage-2 groups (of NG2) whose w2 is fp8 (the first ones)
F8MAX = 14.0    # per-column scale target (e3m4 max normal = 15.5)
ALPHA = 128.0   # constant pre-scale for fp8 w1 (un-done by silu scale)

_cache = {}
last_results = None  # BassKernelResults of the most recent device run


def _np_bf16():
    import ml_dtypes
    return np.dtype(ml_dtypes.bfloat16)


def _np_f8():
    import ml_dtypes
    return np.dtype(ml_dtypes.float8_e3m4)


def _build(C, n1f, n3f, n2g):
    import concourse.mybir as mybir
    import concourse.tile as tile
    from concourse import bacc

    nc = bacc.Bacc(None, target_bir_lowering=False)
    f32 = mybir.dt.float32
    bf16 = mybir.dt.bfloat16
    f8 = mybir.dt.float8e3
    n1b = NG1 - n1f
    n3b = NG1 - n3f
    n2b = NG2 - n2g

    # weights packed on host; each dma_start moves 0.5-1MB with
    # contiguous 4-8KB lines per partition:
    #   w1b [n1b, P, KD, G1, P] bf16 / w1f [n1f, ...] f8e3
    #   w3f [n3f, P, KD, G1, P] f8e3 / w3b [n3b, ...] bf16
    #   w2f [n2g, 2, P, KF//2, G2, P] f8e3 (1MB/DMA)
    #   w2b [n2b, 2, 2, P, KF//4, G2, P] bf16 (1MB/DMA)
    if n1b:
        w1b = nc.declare_dram_parameter("w1b", [n1b, P, KD, G1, P],
                                        bf16, isOutput=False)
    if n1f:
        w1f = nc.declare_dram_parameter("w1f", [n1f, P, KD, G1, P],
                                        f8, isOutput=False)
    if n3f:
        w3f = nc.declare_dram_parameter("w3f", [n3f, P, KD, G1, P],
                                        f8, isOutput=False)
    if n3b:
        w3b = nc.declare_dram_parameter("w3b", [n3b, P, KD, G1, P],
                                        bf16, isOutput=False)
    if n2g:
        w2f = nc.declare_dram_parameter("w2f", [n2g, 2, P, KF // 2, G2, P],
                                        f8, isOutput=False)
    if n2b:
        w2b = nc.declare_dram_parameter("w2b", [n2b, 2, 2, P, KF // 4, G2, P],
                                        bf16, isOutput=False)
    xT = nc.declare_dram_parameter("xT", [P, KD, C], bf16, isOutput=False)
    outT = nc.declare_dram_parameter("outT", [NG2, P, G2, C], f32,
                                     isOutput=True)

    with tile.TileContext(nc) as tc:
        with (
            tc.tile_pool(name="xpool", bufs=1) as xpool,
            tc.tile_pool(name="hpool", bufs=1) as hpool,
            # deep pools: DMA issue must never be gated by PE progress,
            # else the ring goes stop-start and effective rate drops
            tc.tile_pool(name="w1pool", bufs=8) as w1pool,
            tc.tile_pool(name="w3pool", bufs=10) as w3pool,
            tc.tile_pool(name="w2fpool", bufs=4) as w2fpool,
            tc.tile_pool(name="w2bpool", bufs=3) as w2bpool,
            tc.tile_pool(name="psum", bufs=8, space="PSUM") as psum,
            tc.tile_pool(name="spool", bufs=4) as spool,
            tc.tile_pool(name="opool", bufs=2) as opool,
        ):
            xt = xpool.tile([P, KD, C], bf16)
            nc.sync.dma_start(out=xt[:, 0:8, :], in_=xT[:, 0:8, :])
            ht = hpool.tile([P, KF, C], bf16)

            # keep the PE busy from t=0 so HAM is un-throttled by the
            # time the first real matmul issues (~3.4us of activity)
            warm = xpool.tile([P, 256], bf16, name="warm")
            nc.vector.memset(warm[:], 0.0)
            ps_w = psum.tile([P, 256], f32, tag="ps", name="ps_warm")
            for i in range(23):
                nc.tensor.matmul(ps_w[:], warm[:, :P], warm[:],
                                 start=True, stop=True)

            # stage 1: hT[f, t] = silu(w1^T xT) * (w3^T xT)
            for g in range(NG1):
                # bf16 w1 groups first, fp8 last: measured fastest of
                # the three orderings (bf16-first 120.2us, fp8-first
                # 122.2us, parity-interleaved 123.7us)
                w1_is_f8 = g >= n1b
                w3_is_f8 = g < n3f
                dt1 = f8 if w1_is_f8 else bf16
                dt3 = f8 if w3_is_f8 else bf16
                w1src = (w1f[g - n1b] if w1_is_f8 else w1b[g])
                w3src = (w3f[g] if w3_is_f8 else w3b[g - n3f])
                wt1 = w1pool.tile([P, KD, G1, P], dt1, tag="w1")
                wt3 = w3pool.tile([P, KD, G1, P], dt3, tag="w3")
                if g == 0:
                    # first group in quarters so the very first matmuls
                    # start ~1.3us sooner; xt's second half rides in the
                    # middle (needed only from k=8)
                    q = KD // 4
                    for qq in range(4):
                        nc.sync.dma_start(out=wt1[:, qq * q:(qq + 1) * q],
                                          in_=w1src[:, qq * q:(qq + 1) * q])
                        nc.sync.dma_start(out=wt3[:, qq * q:(qq + 1) * q],
                                          in_=w3src[:, qq * q:(qq + 1) * q])
                        if qq == 1:
                            nc.sync.dma_start(out=xt[:, 8:, :],
                                              in_=xT[:, 8:, :])
                elif g == 1:
                    h0 = KD // 2
                    nc.sync.dma_start(out=wt1[:, :h0], in_=w1src[:, :h0])
                    nc.sync.dma_start(out=wt3[:, :h0], in_=w3src[:, :h0])
                    nc.sync.dma_start(out=wt1[:, h0:], in_=w1src[:, h0:])
                    nc.sync.dma_start(out=wt3[:, h0:], in_=w3src[:, h0:])
                else:
                    nc.sync.dma_start(out=wt1[:], in_=w1src)
                    nc.sync.dma_start(out=wt3[:], in_=w3src)
                ps_g = [psum.tile([P, C], f32, tag="ps", name=f"ps_g{g}_{m}")
                        for m in range(G1)]
                ps_u = [psum.tile([P, C], f32, tag="ps", name=f"ps_u{g}_{m}")
                        for m in range(G1)]
                for k in range(KD):
                    st, sp = (k == 0), (k == KD - 1)
                    for m in range(G1):
                        nc.tensor.matmul(ps_g[m][:], wt1[:, k, m, :],
                                         xt[:, k, :], start=st, stop=sp)
                        nc.tensor.matmul(ps_u[m][:], wt3[:, k, m, :],
                                         xt[:, k, :], start=st, stop=sp)
                # fp8 w1 was pre-scaled by ALPHA on host; silu's scale
                # operand un-does it for free: silu(ps * 1/ALPHA)
                sscale = (1.0 / ALPHA) if w1_is_f8 else 1.0
                for m in range(G1):
                    sig = spool.tile([P, C], f32, tag="sig")
                    nc.scalar.activation(sig[:], ps_g[m][:],
                                         mybir.ActivationFunctionType.Silu,
                                         scale=sscale)
                    nc.vector.tensor_tensor(out=ht[:, g * G1 + m, :],
                                            in0=sig[:], in1=ps_u[m][:],
                                            op=mybir.AluOpType.mult)

            # stage 2: outT[d, t] = w2^T @ hT
            for g in range(NG2):
                w2_is_f8 = g < n2g
                dt2 = f8 if w2_is_f8 else bf16
                ps_o = [psum.tile([P, C], f32, tag="ps", name=f"ps_o{g}_{m}")
                        for m in range(G2)]
                wt2s = []
                for kp in range(2):
                    pool2 = w2fpool if w2_is_f8 else w2bpool
                    wt2 = pool2.tile([P, KF // 2, G2, P], dt2, tag="w2")
                    wt2s.append(wt2)
                    if w2_is_f8:
                        if g == 0:
                            # smaller first blocks shrink the stage-
                            # boundary bubble
                            h0 = KF // 4
                            nc.sync.dma_start(out=wt2[:, :h0],
                                              in_=w2f[g, kp, :, :h0])
                            nc.sync.dma_start(out=wt2[:, h0:],
                                              in_=w2f[g, kp, :, h0:])
                        else:
                            nc.sync.dma_start(out=wt2[:], in_=w2f[g, kp])
                    else:
                        # bf16 blocks: 2 DMAs of 1MB each
                        h0 = KF // 4
                        nc.sync.dma_start(out=wt2[:, :h0],
                                          in_=w2b[g - n2g, kp, 0])
                        nc.sync.dma_start(out=wt2[:, h0:],
                                          in_=w2b[g - n2g, kp, 1])
                obuf = opool.tile([P, G2, C], f32, tag="o", name=f"ob{g}")
                if g < NG2 - 1:
                    # k-major: one pass over the weight stream
                    for k in range(KF):
                        kp, kk = divmod(k, KF // 2)
                        st, sp = (k == 0), (k == KF - 1)
                        for m in range(G2):
                            nc.tensor.matmul(ps_o[m][:],
                                             wt2s[kp][:, kk, m, :],
                                             ht[:, k, :], start=st, stop=sp)
                    for m in range(G2):
                        nc.vector.tensor_copy(out=obuf[:, m, :], in_=ps_o[m][:])
                    nc.gpsimd.dma_start(out=outT[g], in_=obuf[:])
                else:
                    # last group m-major: each m-tile finishes early so
                    # its drain+store overlaps the remaining matmuls and
                    # the post-last-matmul tail is one copy + one store
                    for m in range(G2):
                        for k in range(KF):
                            kp, kk = divmod(k, KF // 2)
                            st, sp = (k == 0), (k == KF - 1)
                            nc.tensor.matmul(ps_o[m][:],
                                             wt2s[kp][:, kk, m, :],
                                             ht[:, k, :], start=st, stop=sp)
                        nc.vector.tensor_copy(out=obuf[:, m, :], in_=ps_o[m][:])
                        if m == 1:
                            nc.sync.dma_start(out=outT[g, :, 0:2],
                                              in_=obuf[:, 0:2])
                        elif m == 3:
                            nc.sync.dma_start(out=outT[g, :, 2:],
                                              in_=obuf[:, 2:])

    nc.compile()
    return nc


def _route(x2d, gate_w, top_k):
    """Replicates the reference gate on host: returns (sel [T,k], cw [T,k])."""
    logits = x2d @ gate_w                       # [T, E] fp32
    sel = np.argsort(-logits, axis=-1, kind="stable")[:, :top_k]
    vals = np.take_along_axis(logits, sel, axis=-1)
    m = vals.max(axis=-1, keepdims=True)
    ex = np.exp(vals - m)
    cw = ex / ex.sum(axis=-1, keepdims=True)
    return sel, cw


def _pack_s1(w, dt):
    # [D, ncols] -> [ng, P, KD, G1, P]: block g holds all KD k-tiles of
    # m-tiles g*G1..g*G1+G1-1; one 4-8KB line per partition
    ng = w.shape[1] // (G1 * P)
    r = w.astype(dt).reshape(KD, P, ng, G1, P)
    return np.ascontiguousarray(r.transpose(2, 1, 0, 3, 4))


def _pack_s2(w, dt):
    # [F, ncols] -> [ng, 2, P, KF//2, G2, P]
    ng = w.shape[1] // (G2 * P)
    r = w.astype(dt).reshape(2, KF // 2, P, ng, G2, P)
    return np.ascontiguousarray(r.transpose(3, 0, 2, 1, 4, 5))


def kernel(x, gate_w, w1, w3, w2, top_k):
    from concourse.bass_utils import run_bass_kernel_spmd

    x = np.asarray(x, np.float32)
    gate_w = np.asarray(gate_w, np.float32)
    w1 = np.asarray(w1, np.float32)
    w3 = np.asarray(w3, np.float32)
    w2 = np.asarray(w2, np.float32)
    k = int(top_k)

    x2d = x.reshape(T, D)
    sel, cw = _route(x2d, gate_w, k)

    # token lists per expert
    idx = [np.where((sel == e).any(axis=1))[0] for e in range(E)]
    wgt = []
    for e in range(E):
        m = sel[idx[e]] == e
        wgt.append(cw[idx[e]][m].astype(np.float32))
    counts = np.array([len(i) for i in idx])
    maxc = int(counts.max())
    C = max(128, -(-maxc // 16) * 16)
    n_chunks = 1
    if C > 512:  # capacity overflow: run multiple passes of 512
        C = 512
        n_chunks = -(-maxc // C)

    key = (C, W1_NFP8, W3_NFP8, W2_NFP8G)
    if key not in _cache:
        _cache[key] = _build(C, W1_NFP8, W3_NFP8, W2_NFP8G)
    nc = _cache[key]

    bf16 = _np_bf16()
    f8 = _np_f8()

    n1b = NG1 - W1_NFP8
    nf1 = n1b * G1 * P        # first nf1 columns of w1 stay bf16
    nf3 = W3_NFP8 * G1 * P    # first nf3 columns of w3 are fp8
    nf2 = W2_NFP8G * G2 * P   # first nf2 columns of w2 are fp8

    wpacked = []
    for e in range(E):
        # per-column scale for w3 (all columns, so w2pre rows stay at a
        # uniform magnitude); folds into w2's rows exactly
        s3 = np.abs(w3[e]).max(axis=0) / F8MAX
        w3e = w3[e] / s3[None, :]
        w2pre = w2[e] * s3[:, None]
        # per-column scale for the fp8 part of w2; folds into the host
        # combine exactly
        s2 = np.ones(D, np.float32)
        if nf2:
            s2[:nf2] = np.abs(w2pre[:, :nf2]).max(axis=0) / F8MAX
            w2pre = w2pre / s2[None, :]

        maps = {}
        if n1b:
            maps["w1b"] = _pack_s1(w1[e][:, :nf1], bf16)
        if W1_NFP8:
            w1a = np.clip(w1[e][:, nf1:] * ALPHA, -15.4, 15.4)
            maps["w1f"] = _pack_s1(w1a, f8)
        if W3_NFP8:
            maps["w3f"] = _pack_s1(w3e[:, :nf3], f8)
        if W3_NFP8 < NG1:
            maps["w3b"] = _pack_s1(w3e[:, nf3:], bf16)
        if W2_NFP8G:
            maps["w2f"] = _pack_s2(w2pre[:, :nf2], f8)
        if W2_NFP8G < NG2:
            # bf16 blocks carry an extra half-split axis for 1MB DMAs
            p2 = _pack_s2(w2pre[:, nf2:], bf16)
            ng = p2.shape[0]
            maps["w2b"] = np.ascontiguousarray(
                p2.reshape(ng, 2, P, 2, KF // 4, G2, P).transpose(
                    0, 1, 3, 2, 4, 5, 6))
        wpacked.append((maps, s2))

    out = np.zeros((T, D), np.float32)
    for chunk in range(n_chunks):
        in_maps = []
        for e in range(E):
            ide = idx[e][chunk * C:(chunk + 1) * C]
            xTe = np.zeros((D, C), bf16)
            xTe[:, :len(ide)] = x2d[ide].T.astype(bf16)
            m = dict(wpacked[e][0])
            m["xT"] = np.ascontiguousarray(
                xTe.reshape(KD, P, C).transpose(1, 0, 2))
            in_maps.append(m)
        res = run_bass_kernel_spmd(nc, in_maps, core_ids=list(range(E)))
        global last_results
        last_results = res
        for e in range(E):
            ide = idx[e][chunk * C:(chunk + 1) * C]
            if len(ide) == 0:
                continue
            we = wgt[e][chunk * C:(chunk + 1) * C]
            s2 = wpacked[e][1]
            # outT [NG2, P, G2, C] -> [D, C] with d = g*G2*P + m*P + p
            oTe = res.results[e]["outT"].transpose(0, 2, 1, 3).reshape(D, C)
            out[ide] += we[:, None] * (oTe[:, :len(ide)].T * s2[None, :])

    return out.reshape(B, S, D)


# revision 32
# speedup vs baseline: 1.0120x; 1.0120x over previous
"""MoE top-2 routing kernel for 8 Trainium2 NeuronCores.

Problem (hardcoded shapes): x [64,8,2048] f32, gate_w [2048,8] f32,
w1/w3 [8,2048,4096] f32, w2 [8,4096,2048] f32, top_k=2.

Strategy (expert parallelism):
  - Host computes the gate (512x8 logits, top-2, softmax) exactly as the
    reference does -- ~17 MFLOP, negligible.
  - Tokens are dispatched per expert (gathered + padded to capacity C),
    one expert per NeuronCore.  Each core runs the SwiGLU FFN for its
    expert over its C token slots:
        outT = w2^T @ (silu(w1^T @ xT) * (w3^T @ xT))
    with all matmuls laid out [K, M]/[K, N] so no on-device transposes
    are needed (tokens are the moving free dim).
  - Combine weights are folded into the host-side scatter-add.

Precision plan (rel-err budget 2e-2; predictions from a bit-exact numpy
model of the device pipeline, which has matched HW to 4 digits on every
run so far):
  - activations bf16
  - w3 all fp8 e3m4, per-column scales folded into w2's rows (exact)
  - w2: 2 of 4 output-column groups fp8 e3m4, per-column scales folded
    into the host combine (exact)
  - w1: 8 of 16 column groups fp8 e3m4 with a constant x128 pre-scale
    (clipped at +-15.4); the 1/128 un-scale folds into the silu's
    `scale` operand (exact, free)
  Bytes are balanced so stage 1 (w1+w3, ~21MB) matches its ~64us of PE
  work and stage 2 (w2, ~12.6MB) only slightly exceeds its ~34us.
  Predicted rel err 1.908e-2.

Perf notes (measured):
  - one HWDGE ring (sync) sustains ~360 GB/s streaming; ALL DMAs go on
    it.  The scalar engine must issue NO DMAs: its ACTIVATEs wait on PE
    progress and, via strict FIFO + shared semaphore lanes, would pace
    the whole DMA subsystem at PE speed.
  - deep weight pools so DMA issue is never gated by PE progress.
  - C = tokens padded to a multiple of 16 (>=128): C must stay 16-
    aligned or half the xt k-slices are 8B-misaligned in SBUF and every
    matmul pays ~+15ns streaming (measured at C=140).
  - stage-1 groups of G1=2 m-tiles (4 PSUM banks) and stage-2 groups of
    G2=4 (4 banks): consecutive groups ping-pong PSUM banks.
  - 25 warmup matmuls on zeros keep the PE busy while the first weight
    blocks land, so HAM reaches K=8/8 before real work and never
    re-throttles.
"""

import numpy as np

B, S, D, F, E = 64, 8, 2048, 4096, 8
T = B * S  # 512 tokens
P = 128
KD = D // P   # 16 k-tiles, D contraction
KF = F // P   # 32 k-tiles, F contraction
MF = F // P   # 32 m-tiles, stage 1
MD = D // P   # 16 m-tiles, stage 2
G1 = 2        # stage-1 m-tiles per group (2 gate + 2 up = 4 PSUM banks)
G2 = 4        # stage-2 m-tiles per group (4 PSUM banks)
NG1 = MF // G1   # 16 stage-1 groups
NG2 = MD // G2   # 4 stage-2 groups

W1_NFP8 = 8     # stage-1 groups (of NG1) whose w1 is fp8 (the last ones)
W3_NFP8 = 16    # stage-1 groups (of NG1) whose w3 is fp8
W2_NFP8G = 2    # stage-2 groups (of NG2) whose w2 is fp8 (the first ones)
F8MAX = 14.0    # per-column scale target (e3m4 max normal = 15.5)
ALPHA = 128.0   # constant pre-scale for fp8 w1 (un-done by silu scale)

_cache = {}
last_results = None  # BassKernelResults of the most recent device run


def _np_bf16():
    import ml_dtypes
    return np.dtype(ml_dtypes.bfloat16)


def _np_f8():
    import ml_dtypes
    return np.dtype(ml_dtypes.float8_e3m4)


def _build(C, n1f, n3f, n2g):
    import concourse.mybir as mybir
    import concourse.tile as tile
    from concourse import bacc

    nc = bacc.Bacc(None, target_bir_lowering=False)
    f32 = mybir.dt.float32
    bf16 = mybir.dt.bfloat16
    f8 = mybir.dt.float8e3
    n1b = NG1 - n1f
    n3b = NG1 - n3f
    n2b = NG2 - n2g

    # weights packed on host; each dma_start moves 0.5-1MB with
    # contiguous 4-8KB lines per partition:
    #   w1b [n1b, P, KD, G1, P] bf16 / w1f [n1f, ...] f8e3
    #   w3f [n3f, P, KD, G1, P] f8e3 / w3b [n3b, ...] bf16
    #   w2f [n2g, 2, P, KF//2, G2, P] f8e3 (1MB/DMA)
    #   w2b [n2b, 2, 2, P, KF//4, G2, P] bf16 (1MB/DMA)
    if n1b:
        w1b = nc.declare_dram_parameter("w1b", [n1b, P, KD, G1, P],
                                        bf16, isOutput=False)
    if n1f:
        w1f = nc.declare_dram_parameter("w1f", [n1f, P, KD, G1, P],
                                        f8, isOutput=False)
    if n3f:
        w3f = nc.declare_dram_parameter("w3f", [n3f, P, KD, G1, P],
                                        f8, isOutput=False)
    if n3b:
        w3b = nc.declare_dram_parameter("w3b", [n3b, P, KD, G1, P],
                                        bf16, isOutput=False)
    if n2g:
        w2f = nc.declare_dram_parameter("w2f", [n2g, 2, P, KF // 2, G2, P],
                                        f8, isOutput=False)
    if n2b:
        w2b = nc.declare_dram_parameter("w2b", [n2b, 2, 2, P, KF // 4, G2, P],
                                        bf16, isOutput=False)
    xT = nc.declare_dram_parameter("xT", [P, KD, C], bf16, isOutput=False)
    outT = nc.declare_dram_parameter("outT", [NG2, P, G2, C], f32,
                                     isOutput=True)

    with tile.TileContext(nc) as tc:
        with (
            tc.tile_pool(name="xpool", bufs=1) as xpool,
            tc.tile_pool(name="hpool", bufs=1) as hpool,
            # deep pools: DMA issue must never be gated by PE progress,
            # else the ring goes stop-start and effective rate drops
            tc.tile_pool(name="w1pool", bufs=8) as w1pool,
            tc.tile_pool(name="w3pool", bufs=10) as w3pool,
            tc.tile_pool(name="w2fpool", bufs=4) as w2fpool,
            tc.tile_pool(name="w2bpool", bufs=3) as w2bpool,
            tc.tile_pool(name="psum", bufs=8, space="PSUM") as psum,
            tc.tile_pool(name="spool", bufs=4) as spool,
            tc.tile_pool(name="opool", bufs=2) as opool,
        ):
            xt = xpool.tile([P, KD, C], bf16)
            nc.sync.dma_start(out=xt[:, 0:8, :], in_=xT[:, 0:8, :])
            ht = hpool.tile([P, KF, C], bf16)

            # keep the PE busy from t=0 so HAM is un-throttled by the
            # time the first real matmul issues (~3.4us of activity)
            warm = xpool.tile([P, 256], bf16, name="warm")
            nc.vector.memset(warm[:], 0.0)
            ps_w = psum.tile([P, 256], f32, tag="ps", name="ps_warm")
            for i in range(25):
                nc.tensor.matmul(ps_w[:], warm[:, :P], warm[:],
                                 start=True, stop=True)

            # stage 1: hT[f, t] = silu(w1^T xT) * (w3^T xT)
            for g in range(NG1):
                # bf16 w1 groups first, fp8 last: measured fastest of
                # the three orderings (bf16-first 120.2us, fp8-first
                # 122.2us, parity-interleaved 123.7us)
                w1_is_f8 = g >= n1b
                w3_is_f8 = g < n3f
                dt1 = f8 if w1_is_f8 else bf16
                dt3 = f8 if w3_is_f8 else bf16
                w1src = (w1f[g - n1b] if w1_is_f8 else w1b[g])
                w3src = (w3f[g] if w3_is_f8 else w3b[g - n3f])
                wt1 = w1pool.tile([P, KD, G1, P], dt1, tag="w1")
                wt3 = w3pool.tile([P, KD, G1, P], dt3, tag="w3")
                if g < 2:
                    # first groups: halve the blocks so the first
                    # matmuls can start ~3us sooner; xt's second half
                    # rides between them (needed only from k=8)
                    h0 = KD // 2
                    nc.sync.dma_start(out=wt1[:, :h0], in_=w1src[:, :h0])
                    nc.sync.dma_start(out=wt3[:, :h0], in_=w3src[:, :h0])
                    if g == 0:
                        nc.sync.dma_start(out=xt[:, 8:, :], in_=xT[:, 8:, :])
                    nc.sync.dma_start(out=wt1[:, h0:], in_=w1src[:, h0:])
                    nc.sync.dma_start(out=wt3[:, h0:], in_=w3src[:, h0:])
                else:
                    nc.sync.dma_start(out=wt1[:], in_=w1src)
                    nc.sync.dma_start(out=wt3[:], in_=w3src)
                ps_g = [psum.tile([P, C], f32, tag="ps", name=f"ps_g{g}_{m}")
                        for m in range(G1)]
                ps_u = [psum.tile([P, C], f32, tag="ps", name=f"ps_u{g}_{m}")
                        for m in range(G1)]
                for k in range(KD):
                    st, sp = (k == 0), (k == KD - 1)
                    for m in range(G1):
                        nc.tensor.matmul(ps_g[m][:], wt1[:, k, m, :],
                                         xt[:, k, :], start=st, stop=sp)
                        nc.tensor.matmul(ps_u[m][:], wt3[:, k, m, :],
                                         xt[:, k, :], start=st, stop=sp)
                # fp8 w1 was pre-scaled by ALPHA on host; silu's scale
                # operand un-does it for free: silu(ps * 1/ALPHA)
                sscale = (1.0 / ALPHA) if w1_is_f8 else 1.0
                for m in range(G1):
                    sig = spool.tile([P, C], f32, tag="sig")
                    nc.scalar.activation(sig[:], ps_g[m][:],
                                         mybir.ActivationFunctionType.Silu,
                                         scale=sscale)
                    nc.vector.tensor_tensor(out=ht[:, g * G1 + m, :],
                                            in0=sig[:], in1=ps_u[m][:],
                                            op=mybir.AluOpType.mult)

            # stage 2: outT[d, t] = w2^T @ hT
            for g in range(NG2):
                w2_is_f8 = g < n2g
                dt2 = f8 if w2_is_f8 else bf16
                ps_o = [psum.tile([P, C], f32, tag="ps", name=f"ps_o{g}_{m}")
                        for m in range(G2)]
                wt2s = []
                for kp in range(2):
                    pool2 = w2fpool if w2_is_f8 else w2bpool
                    wt2 = pool2.tile([P, KF // 2, G2, P], dt2, tag="w2")
                    wt2s.append(wt2)
                    if w2_is_f8:
                        if g == 0:
                            # smaller first blocks shrink the stage-
                            # boundary bubble
                            h0 = KF // 4
                            nc.sync.dma_start(out=wt2[:, :h0],
                                              in_=w2f[g, kp, :, :h0])
                            nc.sync.dma_start(out=wt2[:, h0:],
                                              in_=w2f[g, kp, :, h0:])
                        else:
                            nc.sync.dma_start(out=wt2[:], in_=w2f[g, kp])
                    else:
                        # bf16 blocks: 2 DMAs of 1MB each
                        h0 = KF // 4
                        nc.sync.dma_start(out=wt2[:, :h0],
                                          in_=w2b[g - n2g, kp, 0])
                        nc.sync.dma_start(out=wt2[:, h0:],
                                          in_=w2b[g - n2g, kp, 1])
                obuf = opool.tile([P, G2, C], f32, tag="o", name=f"ob{g}")
                if g < NG2 - 1:
                    # k-major: one pass over the weight stream
                    for k in range(KF):
                        kp, kk = divmod(k, KF // 2)
                        st, sp = (k == 0), (k == KF - 1)
                        for m in range(G2):
                            nc.tensor.matmul(ps_o[m][:],
                                             wt2s[kp][:, kk, m, :],
                                             ht[:, k, :], start=st, stop=sp)
                    for m in range(G2):
                        nc.vector.tensor_copy(out=obuf[:, m, :], in_=ps_o[m][:])
                    nc.gpsimd.dma_start(out=outT[g], in_=obuf[:])
                else:
                    # last group m-major: each m-tile finishes early so
                    # its drain+store overlaps the remaining matmuls and
                    # the post-last-matmul tail is one copy + one store
                    for m in range(G2):
                        for k in range(KF):
                            kp, kk = divmod(k, KF // 2)
                            st, sp = (k == 0), (k == KF - 1)
                            nc.tensor.matmul(ps_o[m][:],
                                             wt2s[kp][:, kk, m, :],
                                             ht[:, k, :], start=st, stop=sp)
                        nc.vector.tensor_copy(out=obuf[:, m, :], in_=ps_o[m][:])
                        if m == 1:
                            nc.sync.dma_start(out=outT[g, :, 0:2],
                                              in_=obuf[:, 0:2])
                        elif m == 3:
                            nc.sync.dma_start(out=outT[g, :, 2:],
                                              in_=obuf[:, 2:])

    nc.compile()
    return nc


def _route(x2d, gate_w, top_k):
    """Replicates the reference gate on host: returns (sel [T,k], cw [T,k])."""
    logits = x2d @ gate_w                       # [T, E] fp32
    sel = np.argsort(-logits, axis=-1, kind="stable")[:, :top_k]
    vals = np.take_along_axis(logits, sel, axis=-1)
    m = vals.max(axis=-1, keepdims=True)
    ex = np.exp(vals - m)
    cw = ex / ex.sum(axis=-1, keepdims=True)
    return sel, cw


def _pack_s1(w, dt):
    # [D, ncols] -> [ng, P, KD, G1, P]: block g holds all KD k-tiles of
    # m-tiles g*G1..g*G1+G1-1; one 4-8KB line per partition
    ng = w.shape[1] // (G1 * P)
    r = w.astype(dt).reshape(KD, P, ng, G1, P)
    return np.ascontiguousarray(r.transpose(2, 1, 0, 3, 4))


def _pack_s2(w, dt):
    # [F, ncols] -> [ng, 2, P, KF//2, G2, P]
    ng = w.shape[1] // (G2 * P)
    r = w.astype(dt).reshape(2, KF // 2, P, ng, G2, P)
    return np.ascontiguousarray(r.transpose(3, 0, 2, 1, 4, 5))


def kernel(x, gate_w, w1, w3, w2, top_k):
    from concourse.bass_utils import run_bass_kernel_spmd

    x = np.asarray(x, np.float32)
    gate_w = np.asarray(gate_w, np.float32)
    w1 = np.asarray(w1, np.float32)
    w3 = np.asarray(w3, np.float32)
    w2 = np.asarray(w2, np.float32)
    k = int(top_k)

    x2d = x.reshape(T, D)
    sel, cw = _route(x2d, gate_w, k)

    # token lists per expert
    idx = [np.where((sel == e).any(axis=1))[0] for e in range(E)]
    wgt = []
    for e in range(E):
        m = sel[idx[e]] == e
        wgt.append(cw[idx[e]][m].astype(np.float32))
    counts = np.array([len(i) for i in idx])
    maxc = int(counts.max())
    C = max(128, -(-maxc // 16) * 16)
    n_chunks = 1
    if C > 512:  # capacity overflow: run multiple passes of 512
        C = 512
        n_chunks = -(-maxc // C)

    key = (C, W1_NFP8, W3_NFP8, W2_NFP8G)
    if key not in _cache:
        _cache[key] = _build(C, W1_NFP8, W3_NFP8, W2_NFP8G)
    nc = _cache[key]

    bf16 = _np_bf16()
    f8 = _np_f8()

    n1b = NG1 - W1_NFP8
    nf1 = n1b * G1 * P        # first nf1 columns of w1 stay bf16
    nf3 = W3_NFP8 * G1 * P    # first nf3 columns of w3 are fp8
    nf2 = W2_NFP8G * G2 * P   # first nf2 columns of w2 are fp8

    wpacked = []
    for e in range(E):
        # per-column scale for w3 (all columns, so w2pre rows stay at a
        # uniform magnitude); folds into w2's rows exactly
        s3 = np.abs(w3[e]).max(axis=0) / F8MAX
        w3e = w3[e] / s3[None, :]
        w2pre = w2[e] * s3[:, None]
        # per-column scale for the fp8 part of w2; folds into the host
        # combine exactly
        s2 = np.ones(D, np.float32)
        if nf2:
            s2[:nf2] = np.abs(w2pre[:, :nf2]).max(axis=0) / F8MAX
            w2pre = w2pre / s2[None, :]

        maps = {}
        if n1b:
            maps["w1b"] = _pack_s1(w1[e][:, :nf1], bf16)
        if W1_NFP8:
            w1a = np.clip(w1[e][:, nf1:] * ALPHA, -15.4, 15.4)
            maps["w1f"] = _pack_s1(w1a, f8)
        if W3_NFP8:
            maps["w3f"] = _pack_s1(w3e[:, :nf3], f8)
        if W3_NFP8 < NG1:
            maps["w3b"] = _pack_s1(w3e[:, nf3:], bf16)
        if W2_NFP8G:
            maps["w2f"] = _pack_s2(w2pre[:, :nf2], f8)
        if W2_NFP8G < NG2:
            # bf16 blocks carry an extra half-split axis for 1MB DMAs
            p2 = _pack_s2(w2pre[:, nf2:], bf16)
            ng = p2.shape[0]
            maps["w2b"] = np.ascontiguousarray(
                p2.reshape(ng, 2, P, 2, KF // 4, G2, P).transpose(
                    0, 1, 3, 2, 4, 5, 6))
        wpacked.append((maps, s2))

    out = np.zeros((T, D), np.float32)
    for chunk in range(n_chunks):
        in_maps = []
        for e in range(E):
            ide = idx[e][chunk * C:(chunk + 1) * C]
            xTe = np.zeros((D, C), bf16)
            xTe[:, :len(ide)] = x2d[ide].T.astype(bf16)
            m = dict(wpacked[e][0])
            m["xT"] = np.ascontiguousarray(
                xTe.reshape(KD, P, C).transpose(1, 0, 2))
            in_maps.append(m)
        res = run_bass_kernel_spmd(nc, in_maps, core_ids=list(range(E)))
        global last_results
        last_results = res
        for e in range(E):
            ide = idx[e][chunk * C:(chunk + 1) * C]
            if len(ide) == 0:
                continue
            we = wgt[e][chunk * C:(chunk + 1) * C]
            s2 = wpacked[e][1]
            # outT [NG2, P, G2, C] -> [D, C] with d = g*G2*P + m*P + p
            oTe = res.results[e]["outT"].transpose(0, 2, 1, 3).reshape(D, C)
            out[ide] += we[:, None] * (oTe[:, :len(ide)].T * s2[None, :])

    return out.reshape(B, S, D)
